# revision 24
# baseline (speedup 1.0000x reference)
"""Multi-head cross-attention (B=4, S=2048, D=1024, H=16) on 8 Trainium2 cores.

Sharding: hybrid data/tensor parallel. Core c handles batch b = c//2 and
head-group g = c%2 (8 of the 16 heads, i.e. 512 of the 1024 q/k/v dims).
Each core computes a partial out-projection over its 512 attention dims;
the host sums the two partials per batch.

v3: software-pipelined single-core schedule. The attention stream
(logits -> exp -> AV) is ACT-bound per chunk, so projection matmul
"filler" units are interleaved into the attention blocks to keep the
PE dense:
  - prologue: K-proj(m=0), Q-proj(mt=0), V(st=0)
  - block (mt0,h0,qh0): V(st=1..15) emitted just-in-time
  - later blocks: K(m=mt+1) / Q(mt+1) fillers; out-proj n=0 fillers in
    the last two blocks; rest of out-proj in the epilogue
PSUM: lg ring x2 (4 banks) + av x1 (2) + proj x1 (2). The av psum is
freed immediately by a copy to SBUF; normalization (reciprocal +
partition broadcast + multiply) runs off the critical path from SBUF.
Within a block, lg(kc) matmuls are emitted before av(kc-1) so the PE
never head-of-line blocks on the exp of the current chunk.
"""

import numpy as np

import concourse.bacc as bacc
import concourse.mybir as mybir
from concourse import tile
from concourse.bass_utils import run_bass_kernel_spmd

F32 = mybir.dt.float32
F16 = mybir.dt.float16
AF = mybir.ActivationFunctionType

B, S, D = 4, 2048, 1024
H, HD = 16, 64
NCORES = 8
NH = 8          # heads per core
OD = NH * HD    # 512 attention dims per core
P = 128
NDC = D // P    # 8 d-chunks
NKC = S // P    # 16 key chunks
NEG = -1.0e30

_cache = {}


def _build():
    from contextlib import ExitStack

    nc = bacc.Bacc(None, target_bir_lowering=False, debug=False)

    x_t = nc.dram_tensor("x_t", [D, S], F16, kind="ExternalInput").ap()
    mem_t = nc.dram_tensor("mem_t", [D, S], F16, kind="ExternalInput").ap()
    wq_t = nc.dram_tensor("wq_t", [D, OD], F16, kind="ExternalInput").ap()
    wk_t = nc.dram_tensor("wk_t", [D, OD], F16, kind="ExternalInput").ap()
    wv_t = nc.dram_tensor("wv_t", [D, OD], F16, kind="ExternalInput").ap()
    wo_t = nc.dram_tensor("wo_t", [OD, D], F16, kind="ExternalInput").ap()
    bq_s = nc.dram_tensor("bq_s", [P, OD // P], F32, kind="ExternalInput").ap()
    bk_s = nc.dram_tensor("bk_s", [P, OD // P], F32, kind="ExternalInput").ap()
    bo_s = nc.dram_tensor("bo_s", [P, D // P], F32, kind="ExternalInput").ap()
    maskb = nc.dram_tensor("maskb", [P, NKC], F32, kind="ExternalInput").ap()
    out_t = nc.dram_tensor("out_t", [D, S], F16, kind="ExternalOutput").ap()

    x_c = x_t.rearrange("(c p) s -> c p s", p=P)
    m_c = mem_t.rearrange("(c p) s -> c p s", p=P)
    wq_c = wq_t.rearrange("(c p) o -> c p o", p=P)
    wk_c = wk_t.rearrange("(c p) o -> c p o", p=P)
    wv_c = wv_t.rearrange("(c p) o -> c p o", p=P)
    wo_c = wo_t.rearrange("(c p) o -> c p o", p=P)

    with tile.TileContext(nc) as tc, ExitStack() as ctx:
        q_pool = ctx.enter_context(tc.tile_pool(name="qt", bufs=1))
        k_pool = ctx.enter_context(tc.tile_pool(name="kt", bufs=1))
        v_pool = ctx.enter_context(tc.tile_pool(name="va", bufs=1))
        a_pool = ctx.enter_context(tc.tile_pool(name="at", bufs=1))
        c_pool = ctx.enter_context(tc.tile_pool(name="cst", bufs=1))
        w_pool = ctx.enter_context(tc.tile_pool(name="wt", bufs=1))
        e_pool = ctx.enter_context(tc.tile_pool(name="es", bufs=6))
        n_pool = ctx.enter_context(tc.tile_pool(name="nrm", bufs=1))
        o_pool = ctx.enter_context(tc.tile_pool(name="ev", bufs=2))
        s_pool = ctx.enter_context(tc.tile_pool(name="avs", bufs=2))
        lg_pool = ctx.enter_context(tc.tile_pool(name="plg", bufs=2, space="PSUM"))
        av_pool = ctx.enter_context(tc.tile_pool(name="pav", bufs=1, space="PSUM"))
        pj_pool = ctx.enter_context(tc.tile_pool(name="ppj", bufs=1, space="PSUM"))
        xm_pool = ctx.enter_context(tc.tile_pool(name="xm", bufs=32))

        # ---- weight / input DMAs: spread across queues, first-needed first
        wk_sb = [w_pool.tile([P, OD], F16, tag="wk", name=f"wk{i}", bufs=NDC)
                 for i in range(NDC)]
        for i in range(NDC):
            nc.sync.dma_start(out=wk_sb[i][:], in_=wk_c[i])

        # queue plan (first-needed first):
        #   sync:   wk, wq, wv, wo
        #   gpsimd: m n0, x n0 (even i), x n1
        #   scalar: mask, bk, bq, m n1, x n0 (odd i), bo
        m_sb = [[xm_pool.tile([P, 1024], F16, tag="xm", name=f"m{n}_{i}")
                 for i in range(NDC)] for n in range(2)]
        x_sb = [[xm_pool.tile([P, 1024], F16, tag="xm", name=f"x{n}_{i}")
                 for i in range(NDC)] for n in range(2)]
        for i in range(NDC):
            nc.gpsimd.dma_start(out=m_sb[0][i][:], in_=m_c[i, :, 0:1024])
        bq_sb = c_pool.tile([P, OD // P], F32, tag="bq")
        bk_sb = c_pool.tile([P, OD // P], F32, tag="bk")
        bo_sb = c_pool.tile([P, D // P], F32, tag="bo")
        mk_sb = c_pool.tile([P, NKC], F32, tag="mk")
        nc.scalar.dma_start(out=mk_sb[:], in_=maskb[:])
        nc.scalar.dma_start(out=bk_sb[:], in_=bk_s[:])
        nc.scalar.dma_start(out=bq_sb[:], in_=bq_s[:])
        for i in range(NDC):
            nc.scalar.dma_start(out=m_sb[1][i][:], in_=m_c[i, :, 1024:2048])

        wq_sb = [w_pool.tile([P, OD], F16, tag="wq", name=f"wq{i}", bufs=NDC)
                 for i in range(NDC)]
        for i in range(NDC):
            nc.sync.dma_start(out=wq_sb[i][:], in_=wq_c[i])
        wv_sb = [w_pool.tile([P, OD], F16, tag="wv", name=f"wv{i}", bufs=NDC)
                 for i in range(NDC)]
        for i in range(NDC):
            nc.sync.dma_start(out=wv_sb[i][:], in_=wv_c[i])

        for i in range(NDC):
            eng = nc.gpsimd if i % 2 == 0 else nc.scalar
            eng.dma_start(out=x_sb[0][i][:], in_=x_c[i, :, 0:1024])
        for i in range(NDC):
            nc.gpsimd.dma_start(out=x_sb[1][i][:], in_=x_c[i, :, 1024:2048])
        nc.scalar.dma_start(out=bo_sb[:], in_=bo_s[:])

        wo_sb = [w_pool.tile([P, D], F16, tag="wo", name=f"wo{i}", bufs=OD // P)
                 for i in range(OD // P)]
        for i in range(OD // P):
            nc.sync.dma_start(out=wo_sb[i][:], in_=wo_c[i])

        # ---- persistent tiles ----
        qT = [q_pool.tile([P, S], F16, tag=f"q{m}", name=f"q{m}")
              for m in range(OD // P)]
        # packed K: rows 0:64 = head 2m, rows 64:128 = head 2m+1; the
        # logits matmul contracts only the 64 rows of its head
        kP = [k_pool.tile([P, S], F16, tag=f"k{m}", name=f"k{m}")
              for m in range(OD // P)]
        ones_f = c_pool.tile([P, NH], F32, tag="onef")
        nc.vector.memset(ones_f[:], 1.0)
        ones_r = c_pool.tile([P, NH], F16, tag="oner")
        nc.vector.tensor_copy(ones_r[:], ones_f[:])
        v_aug = [v_pool.tile([P, 9, 65], F16, tag=f"v{st}", name=f"v{st}")
                 for st in range(NKC)]
        for st in range(NKC):
            nc.vector.memset(v_aug[st][:, 8, :], 0.0)
        # attn tiles split by query half for precise out-proj deps
        attn = [[a_pool.tile([P, 1024], F16, tag=f"a{n}_{m}", name=f"a{n}_{m}")
                 for m in range(OD // P)] for n in range(2)]

        # ---------- filler unit machinery ----------
        def kproj_units(m):
            units = []
            for n in range(2):
                state = {}
                csl = slice(n * 1024, (n + 1) * 1024)
                for part in range(4):
                    def u(m=m, n=n, part=part, state=state, csl=csl):
                        if part == 0:
                            state["ps"] = pj_pool.tile(
                                [P, 1024], F32, tag="pj", name=f"pk{m}{n}")
                        ps = state["ps"]
                        for i in range(part * 2, part * 2 + 2):
                            for j in range(2):
                                nc.tensor.matmul(
                                    ps[:, j * 512:(j + 1) * 512],
                                    wk_sb[i][:, m * P:(m + 1) * P],
                                    m_sb[n][i][:, j * 512:(j + 1) * 512],
                                    start=(i == 0), stop=(i == NDC - 1),
                                )
                        if part == 3:
                            nc.vector.tensor_scalar_add(
                                kP[m][:, csl], ps[:], bk_sb[:, m:m + 1])
                    units.append(u)
            return units

        def qproj_units(mt, ns=(0, 1)):
            units = []
            for n in ns:
                state = {}
                csl = slice(n * 1024, (n + 1) * 1024)
                for part in range(4):
                    def u(mt=mt, n=n, part=part, state=state, csl=csl):
                        if part == 0:
                            state["ps"] = pj_pool.tile(
                                [P, 1024], F32, tag="pj", name=f"pq{mt}{n}")
                        ps = state["ps"]
                        for i in range(part * 2, part * 2 + 2):
                            for j in range(2):
                                nc.tensor.matmul(
                                    ps[:, j * 512:(j + 1) * 512],
                                    wq_sb[i][:, mt * P:(mt + 1) * P],
                                    x_sb[n][i][:, j * 512:(j + 1) * 512],
                                    start=(i == 0), stop=(i == NDC - 1),
                                )
                        if part == 3:
                            nc.vector.tensor_scalar_add(
                                qT[mt][:, csl], ps[:], bq_sb[:, mt:mt + 1])
                    units.append(u)
            return units

        def vproj_units(st):
            units = []
            state = {}
            n, sc = divmod(st, 8)
            for part in range(2):
                def u(st=st, n=n, sc=sc, part=part, state=state):
                    if part == 0:
                        state["ps"] = pj_pool.tile(
                            [P, 1024], F32, tag="pj", name=f"pv{st}")
                    ps = state["ps"]
                    for i in range(part * 4, part * 4 + 4):
                        nc.tensor.matmul(
                            ps[:, 0:OD], m_sb[n][i][:, sc * P:(sc + 1) * P],
                            wv_sb[i][:],
                            start=(i == 0), stop=(i == NDC - 1),
                        )
                    if part == 1:
                        nc.vector.tensor_copy(
                            v_aug[st][:, 0:NH, 0:64],
                            ps[:, 0:OD].rearrange("p (h d) -> p h d", h=NH),
                        )
                        nc.vector.tensor_copy(
                            v_aug[st][:, 0:NH, 64:65], ones_r[:].unsqueeze(2))
                units.append(u)
            return units

        def oproj_units(m, n, pool=None):
            units = []
            state = {}
            csl = slice(n * 1024, (n + 1) * 1024)
            psum = pool if pool is not None else pj_pool
            tag = {id(pj_pool): "pj", id(lg_pool): "lg",
                   id(av_pool): "av"}[id(psum)]
            for part in range(2):
                def u(m=m, n=n, part=part, state=state, csl=csl,
                      psum=psum, tag=tag):
                    if part == 0:
                        state["ps"] = psum.tile(
                            [P, 1024], F32, tag=tag, name=f"po{m}{n}")
                    ps = state["ps"]
                    for i in range(part * 2, part * 2 + 2):
                        for j in range(2):
                            nc.tensor.matmul(
                                ps[:, j * 512:(j + 1) * 512],
                                wo_sb[i][:, m * P:(m + 1) * P],
                                attn[n][i][:, j * 512:(j + 1) * 512],
                                start=(i == 0), stop=(i == OD // P - 1),
                            )
                    if part == 1:
                        ev = o_pool.tile([P, 1024], F16, tag="ev")
                        nc.vector.tensor_scalar_add(
                            ev[:], ps[:], bo_sb[:, m:m + 1])
                        nc.sync.dma_start(
                            out=out_t[m * P:(m + 1) * P, csl], in_=ev[:])
                units.append(u)
            return units

        fillers = []   # drained inside attention blocks

        def attention_block(mt, h, qh, jit_v=False, rate=4):
            """One (head, query-half) attention block, software pipelined.

            jit_v: emit V-projection units just-in-time (first block only).
            rate: drain one filler unit every `rate` chunks.
            """
            ro = 64 * (h % 2)
            av = av_pool.tile([P, 1024], F32, tag="av", name="av")
            es_tiles = {}
            for kc in range(NKC):
                if jit_v and kc + 1 < NKC:
                    for u in vproj_units(kc + 1):
                        u()
                lg = lg_pool.tile([P, 1024], F32, tag="lg", name="lg")
                for j in range(2):
                    nc.tensor.matmul(
                        lg[:, j * 512:(j + 1) * 512],
                        kP[mt][ro:ro + 64, kc * P:(kc + 1) * P],
                        qT[mt][ro:ro + 64, qh * 1024 + j * 512:
                               qh * 1024 + (j + 1) * 512],
                        start=True, stop=True,
                    )
                es = e_pool.tile([P, 1024], F16, tag="es")
                nc.scalar.activation(
                    es[:], lg[:], AF.Exp,
                    bias=mk_sb[:, kc:kc + 1], scale=0.125,
                )
                es_tiles[kc] = es
                # AV for the previous chunk (keeps PE ahead of ACT)
                if kc >= 1:
                    _av_mm(av, h, kc - 1, es_tiles.pop(kc - 1))
                if (not jit_v) and fillers and kc % rate == rate - 1:
                    fillers.pop(0)[1]()
            _av_mm(av, h, NKC - 1, es_tiles.pop(NKC - 1), last=True)
            # free the av psum quickly, normalize from SBUF.
            # NB: reciprocal_approx_fast (custom DVE op) only works on APs
            # based at partition 0 — stage the denominator row there first.
            den = n_pool.tile([1, 1024], F32, tag="dn")
            avs = s_pool.tile([64, 1024], F32, tag="avs")
            rec = n_pool.tile([1, 1024], F32, tag="r0")
            bc = n_pool.tile([64, 1024], F32, tag="bc")
            nc.vector.tensor_copy(den[:], av[64:65, :])
            nc.vector.reciprocal_approx_fast(rec[:], den[:])
            nc.vector.tensor_copy(avs[:], av[0:64, :])
            nc.gpsimd.partition_broadcast(bc[:], rec[:])
            nc.vector.tensor_mul(
                attn[qh][mt][ro:ro + 64, :], avs[:], bc[:])

        def _av_mm(av, h, kc, es, last=False):
            va_flat = v_aug[kc][:].rearrange("p h d -> p (h d)")
            for j in range(2):
                nc.tensor.matmul(
                    av[:, j * 512:(j + 1) * 512],
                    va_flat[:, 65 * h:65 * h + 128],
                    es[:, j * 512:(j + 1) * 512],
                    start=(kc == 0), stop=last,
                )

        # ---------- prologue: K(m0), Q(mt0 n=0), V(st0) dense ----------
        for u in kproj_units(0):
            u()
        for u in qproj_units(0, ns=(0,)):
            u()
        for u in vproj_units(0):
            u()

        # ---------- attention blocks ----------
        # first block carries the V projection just-in-time
        attention_block(0, 0, 0, jit_v=True)
        # queue K/Q for later head-pairs as fillers; tag = the (mt, qh)
        # key before which the unit must have been emitted (barrier below)
        for u in qproj_units(0, ns=(1,)):
            fillers.append((1, u))
        for m in range(1, OD // P):
            for u in kproj_units(m):
                fillers.append((2 * m, u))
            for u in qproj_units(m, ns=(0,)):
                fillers.append((2 * m, u))
            for u in qproj_units(m, ns=(1,)):
                fillers.append((2 * m + 1, u))

        order = []
        for mt in range(OD // P):
            for qh in range(2):
                for h in (2 * mt, 2 * mt + 1):
                    if (mt, h, qh) != (0, 0, 0):
                        order.append((mt, h, qh))

        for mt, h, qh in order:
            # barrier: everything this block needs must be emitted first
            while fillers and fillers[0][0] <= 2 * mt + qh:
                fillers.pop(0)[1]()
            # out-proj n=0 becomes available once all qh=0 attn written
            if (mt, h, qh) == (3, 6, 1):
                for m in range(D // P):
                    for u in oproj_units(m, 0):
                        fillers.append((4, u))
            rate = 2 if (mt, qh) == (3, 1) else (4 if fillers else NKC + 1)
            attention_block(mt, h, qh, rate=rate)

        # ---------- epilogue ----------
        # drain leftovers, then out-proj n=1; rotate groups across the
        # three free psum pools, part0s first, so earlier groups' matmuls
        # cover the final normalize chain and each psum->ev->store close
        while fillers:
            fillers.pop(0)[1]()
        pools = [pj_pool, lg_pool, av_pool]
        groups = [oproj_units(m, 1, pool=pools[m % 3])
                  for m in range(D // P)]
        for g in range(0, D // P, 3):
            trio = groups[g:g + 3]
            for t in trio:
                t[0]()
            for t in trio:
                t[1]()

    nc.compile()
    return nc


def _prep_inputs(x, memory, mask, wq, bq, wk, bk, wv, bv, wo, bo):
    f = np.float32
    h = np.float16
    wqT = np.ascontiguousarray(wq.T, dtype=f)
    wkT = np.ascontiguousarray(wk.T, dtype=f)
    wvT = np.ascontiguousarray(wv.T, dtype=f)
    woT = np.ascontiguousarray(wo.T, dtype=f)
    bo_eff = (bo.astype(f) + wo.astype(f) @ bv.astype(f))
    zeros_bo = np.zeros_like(bo_eff)
    in_maps = []
    for c in range(NCORES):
        b, g = divmod(c, 2)
        sl = slice(g * OD, (g + 1) * OD)
        bo_c = bo_eff if g == 0 else zeros_bo
        in_maps.append({
            "x_t": np.ascontiguousarray(x[b].T, dtype=h),
            "mem_t": np.ascontiguousarray(memory[b].T, dtype=h),
            "wq_t": np.ascontiguousarray(wqT[:, sl]).astype(h),
            "wk_t": np.ascontiguousarray(wkT[:, sl]).astype(h),
            "wv_t": np.ascontiguousarray(wvT[:, sl]).astype(h),
            "wo_t": np.ascontiguousarray(woT[sl, :]).astype(h),
            "bq_s": np.ascontiguousarray(bq[sl].astype(f).reshape(OD // P, P).T),
            "bk_s": np.ascontiguousarray(bk[sl].astype(f).reshape(OD // P, P).T),
            "bo_s": np.ascontiguousarray(bo_c.reshape(D // P, P).T),
            "maskb": np.ascontiguousarray(
                np.where(mask[b], np.float32(NEG), np.float32(0.0))
                .astype(f).reshape(NKC, P).T),
        })
    return in_maps


def kernel(x, memory, mask, wq, bq, wk, bk, wv, bv, wo, bo, **run_kwargs):
    x = np.asarray(x, dtype=np.float32)
    memory = np.asarray(memory, dtype=np.float32)
    mask = np.asarray(mask)
    if "nc" not in _cache:
        _cache["nc"] = _build()
    nc = _cache["nc"]
    in_maps = _prep_inputs(x, memory, mask, wq, bq, wk, bk, wv, bv, wo, bo)
    res = run_bass_kernel_spmd(nc, in_maps, list(range(NCORES)), **run_kwargs)
    out = np.empty((B, S, D), dtype=np.float32)
    for b in range(B):
        part = (res.results[2 * b]["out_t"].astype(np.float32)
                + res.results[2 * b + 1]["out_t"].astype(np.float32))
        out[b] = part.T
    if run_kwargs:
        _cache["last_results"] = res
    return out


# revision 27
# speedup vs baseline: 1.0500x; 1.0500x over previous
"""Multi-head cross-attention (B=4, S=2048, D=1024, H=16) on 8 Trainium2 cores.

Sharding: hybrid data/tensor parallel. Core c handles batch b = c//2 and
head-group g = c%2 (8 of the 16 heads, i.e. 512 of the 1024 q/k/v dims).
Each core computes a partial out-projection over its 512 attention dims;
the host sums the two partials per batch.

v3: software-pipelined single-core schedule. The attention stream
(logits -> exp -> AV) is ACT-bound per chunk, so projection matmul
"filler" units are interleaved into the attention blocks to keep the
PE dense:
  - prologue: K-proj(m=0), Q-proj(mt=0), V(st=0)
  - block (mt0,h0,qh0): V(st=1..15) emitted just-in-time
  - later blocks: K(m=mt+1) / Q(mt+1) fillers; out-proj n=0 fillers in
    the last two blocks; rest of out-proj in the epilogue
PSUM: lg ring x2 (4 banks) + av x1 (2) + proj x1 (2). The av psum is
freed immediately by a copy to SBUF; normalization (reciprocal +
partition broadcast + multiply) runs off the critical path from SBUF.
Within a block, lg(kc) matmuls are emitted before av(kc-1) so the PE
never head-of-line blocks on the exp of the current chunk.
"""

import numpy as np

import concourse.bacc as bacc
import concourse.mybir as mybir
from concourse import tile
from concourse.bass_utils import run_bass_kernel_spmd

F32 = mybir.dt.float32
F16 = mybir.dt.float16
AF = mybir.ActivationFunctionType

B, S, D = 4, 2048, 1024
H, HD = 16, 64
NCORES = 8
NH = 8          # heads per core
OD = NH * HD    # 512 attention dims per core
P = 128
NDC = D // P    # 8 d-chunks
NKC = S // P    # 16 key chunks
NEG = -1.0e30

_cache = {}


def _build():
    from contextlib import ExitStack

    nc = bacc.Bacc(None, target_bir_lowering=False, debug=False)

    x_t = nc.dram_tensor("x_t", [D, S], F16, kind="ExternalInput").ap()
    mem_t = nc.dram_tensor("mem_t", [D, S], F16, kind="ExternalInput").ap()
    wq_t = nc.dram_tensor("wq_t", [D, OD], F16, kind="ExternalInput").ap()
    wk_t = nc.dram_tensor("wk_t", [D, OD], F16, kind="ExternalInput").ap()
    wv_t = nc.dram_tensor("wv_t", [D, OD], F16, kind="ExternalInput").ap()
    wo_t = nc.dram_tensor("wo_t", [OD, D], F16, kind="ExternalInput").ap()
    bq_s = nc.dram_tensor("bq_s", [P, OD // P], F32, kind="ExternalInput").ap()
    bk_s = nc.dram_tensor("bk_s", [P, OD // P], F32, kind="ExternalInput").ap()
    bo_s = nc.dram_tensor("bo_s", [P, D // P], F32, kind="ExternalInput").ap()
    maskb = nc.dram_tensor("maskb", [P, NKC], F32, kind="ExternalInput").ap()
    out_t = nc.dram_tensor("out_t", [D, S], F16, kind="ExternalOutput").ap()

    x_c = x_t.rearrange("(c p) s -> c p s", p=P)
    m_c = mem_t.rearrange("(c p) s -> c p s", p=P)
    wq_c = wq_t.rearrange("(c p) o -> c p o", p=P)
    wk_c = wk_t.rearrange("(c p) o -> c p o", p=P)
    wv_c = wv_t.rearrange("(c p) o -> c p o", p=P)
    wo_c = wo_t.rearrange("(c p) o -> c p o", p=P)

    with tile.TileContext(nc) as tc, ExitStack() as ctx:
        q_pool = ctx.enter_context(tc.tile_pool(name="qt", bufs=1))
        k_pool = ctx.enter_context(tc.tile_pool(name="kt", bufs=1))
        v_pool = ctx.enter_context(tc.tile_pool(name="va", bufs=1))
        a_pool = ctx.enter_context(tc.tile_pool(name="at", bufs=1))
        c_pool = ctx.enter_context(tc.tile_pool(name="cst", bufs=1))
        w_pool = ctx.enter_context(tc.tile_pool(name="wt", bufs=1))
        e_pool = ctx.enter_context(tc.tile_pool(name="es", bufs=6))
        n_pool = ctx.enter_context(tc.tile_pool(name="nrm", bufs=1))
        o_pool = ctx.enter_context(tc.tile_pool(name="ev", bufs=2))
        s_pool = ctx.enter_context(tc.tile_pool(name="avs", bufs=2))
        lg_pool = ctx.enter_context(tc.tile_pool(name="plg", bufs=2, space="PSUM"))
        av_pool = ctx.enter_context(tc.tile_pool(name="pav", bufs=1, space="PSUM"))
        pj_pool = ctx.enter_context(tc.tile_pool(name="ppj", bufs=1, space="PSUM"))
        xm_pool = ctx.enter_context(tc.tile_pool(name="xm", bufs=32))

        # ---- weight / input DMAs: spread across queues, first-needed first
        wk_sb = [w_pool.tile([P, OD], F16, tag="wk", name=f"wk{i}", bufs=NDC)
                 for i in range(NDC)]
        for i in range(NDC):
            nc.sync.dma_start(out=wk_sb[i][:], in_=wk_c[i])

        # queue plan (first-needed first):
        #   sync:   wk, wq, wv, wo
        #   gpsimd: m n0, x n0 (even i), x n1
        #   scalar: mask, bk, bq, m n1, x n0 (odd i), bo
        m_sb = [[xm_pool.tile([P, 1024], F16, tag="xm", name=f"m{n}_{i}")
                 for i in range(NDC)] for n in range(2)]
        x_sb = [[xm_pool.tile([P, 1024], F16, tag="xm", name=f"x{n}_{i}")
                 for i in range(NDC)] for n in range(2)]
        for i in range(NDC):
            nc.gpsimd.dma_start(out=m_sb[0][i][:], in_=m_c[i, :, 0:1024])
        bq_sb = c_pool.tile([P, OD // P], F32, tag="bq")
        bk_sb = c_pool.tile([P, OD // P], F32, tag="bk")
        bo_sb = c_pool.tile([P, D // P], F32, tag="bo")
        mk_sb = c_pool.tile([P, NKC], F32, tag="mk")
        nc.scalar.dma_start(out=mk_sb[:], in_=maskb[:])
        nc.scalar.dma_start(out=bk_sb[:], in_=bk_s[:])
        nc.scalar.dma_start(out=bq_sb[:], in_=bq_s[:])
        for i in range(NDC):
            nc.scalar.dma_start(out=m_sb[1][i][:], in_=m_c[i, :, 1024:2048])

        wq_sb = [w_pool.tile([P, OD], F16, tag="wq", name=f"wq{i}", bufs=NDC)
                 for i in range(NDC)]
        for i in range(NDC):
            nc.sync.dma_start(out=wq_sb[i][:], in_=wq_c[i])
        wv_sb = [w_pool.tile([P, OD], F16, tag="wv", name=f"wv{i}", bufs=NDC)
                 for i in range(NDC)]
        for i in range(NDC):
            nc.sync.dma_start(out=wv_sb[i][:], in_=wv_c[i])

        for i in range(NDC):
            eng = nc.gpsimd if i % 2 == 0 else nc.scalar
            eng.dma_start(out=x_sb[0][i][:], in_=x_c[i, :, 0:1024])
        for i in range(NDC):
            nc.gpsimd.dma_start(out=x_sb[1][i][:], in_=x_c[i, :, 1024:2048])
        nc.scalar.dma_start(out=bo_sb[:], in_=bo_s[:])

        wo_sb = [w_pool.tile([P, D], F16, tag="wo", name=f"wo{i}", bufs=OD // P)
                 for i in range(OD // P)]
        for i in range(OD // P):
            nc.sync.dma_start(out=wo_sb[i][:], in_=wo_c[i])

        # ---- persistent tiles ----
        qT = [q_pool.tile([P, S], F16, tag=f"q{m}", name=f"q{m}")
              for m in range(OD // P)]
        # packed K: rows 0:64 = head 2m, rows 64:128 = head 2m+1; the
        # logits matmul contracts only the 64 rows of its head
        kP = [k_pool.tile([P, S], F16, tag=f"k{m}", name=f"k{m}")
              for m in range(OD // P)]
        ones_f = c_pool.tile([P, NH], F32, tag="onef")
        nc.vector.memset(ones_f[:], 1.0)
        ones_r = c_pool.tile([P, NH], F16, tag="oner")
        nc.vector.tensor_copy(ones_r[:], ones_f[:])
        v_aug = [v_pool.tile([P, 9, 65], F16, tag=f"v{st}", name=f"v{st}")
                 for st in range(NKC)]
        for st in range(NKC):
            nc.vector.memset(v_aug[st][:, 8, :], 0.0)
        # attn tiles split by query half for precise out-proj deps
        attn = [[a_pool.tile([P, 1024], F16, tag=f"a{n}_{m}", name=f"a{n}_{m}")
                 for m in range(OD // P)] for n in range(2)]

        # ---------- filler unit machinery ----------
        def kproj_units(m, ns=(0, 1)):
            units = []
            for n in ns:
                state = {}
                csl = slice(n * 1024, (n + 1) * 1024)
                for part in range(4):
                    def u(m=m, n=n, part=part, state=state, csl=csl):
                        if part == 0:
                            state["ps"] = pj_pool.tile(
                                [P, 1024], F32, tag="pj", name=f"pk{m}{n}")
                        ps = state["ps"]
                        for i in range(part * 2, part * 2 + 2):
                            for j in range(2):
                                nc.tensor.matmul(
                                    ps[:, j * 512:(j + 1) * 512],
                                    wk_sb[i][:, m * P:(m + 1) * P],
                                    m_sb[n][i][:, j * 512:(j + 1) * 512],
                                    start=(i == 0), stop=(i == NDC - 1),
                                )
                        if part == 3:
                            nc.vector.tensor_scalar_add(
                                kP[m][:, csl], ps[:], bk_sb[:, m:m + 1])
                    units.append(u)
            return units

        def qproj_units(mt, ns=(0, 1)):
            units = []
            for n in ns:
                state = {}
                csl = slice(n * 1024, (n + 1) * 1024)
                for part in range(4):
                    def u(mt=mt, n=n, part=part, state=state, csl=csl):
                        if part == 0:
                            state["ps"] = pj_pool.tile(
                                [P, 1024], F32, tag="pj", name=f"pq{mt}{n}")
                        ps = state["ps"]
                        for i in range(part * 2, part * 2 + 2):
                            for j in range(2):
                                nc.tensor.matmul(
                                    ps[:, j * 512:(j + 1) * 512],
                                    wq_sb[i][:, mt * P:(mt + 1) * P],
                                    x_sb[n][i][:, j * 512:(j + 1) * 512],
                                    start=(i == 0), stop=(i == NDC - 1),
                                )
                        if part == 3:
                            nc.vector.tensor_scalar_add(
                                qT[mt][:, csl], ps[:], bq_sb[:, mt:mt + 1])
                    units.append(u)
            return units

        def vproj_units(st):
            units = []
            state = {}
            n, sc = divmod(st, 8)
            for part in range(2):
                def u(st=st, n=n, sc=sc, part=part, state=state):
                    if part == 0:
                        state["ps"] = pj_pool.tile(
                            [P, 1024], F32, tag="pj", name=f"pv{st}")
                    ps = state["ps"]
                    for i in range(part * 4, part * 4 + 4):
                        nc.tensor.matmul(
                            ps[:, 0:OD], m_sb[n][i][:, sc * P:(sc + 1) * P],
                            wv_sb[i][:],
                            start=(i == 0), stop=(i == NDC - 1),
                        )
                    if part == 1:
                        nc.vector.tensor_copy(
                            v_aug[st][:, 0:NH, 0:64],
                            ps[:, 0:OD].rearrange("p (h d) -> p h d", h=NH),
                        )
                        nc.vector.tensor_copy(
                            v_aug[st][:, 0:NH, 64:65], ones_r[:].unsqueeze(2))
                units.append(u)
            return units

        def oproj_units(m, n, pool=None):
            units = []
            state = {}
            csl = slice(n * 1024, (n + 1) * 1024)
            psum = pool if pool is not None else pj_pool
            tag = {id(pj_pool): "pj", id(lg_pool): "lg",
                   id(av_pool): "av"}[id(psum)]
            for part in range(2):
                def u(m=m, n=n, part=part, state=state, csl=csl,
                      psum=psum, tag=tag):
                    if part == 0:
                        state["ps"] = psum.tile(
                            [P, 1024], F32, tag=tag, name=f"po{m}{n}")
                    ps = state["ps"]
                    for i in range(part * 2, part * 2 + 2):
                        for j in range(2):
                            nc.tensor.matmul(
                                ps[:, j * 512:(j + 1) * 512],
                                wo_sb[i][:, m * P:(m + 1) * P],
                                attn[n][i][:, j * 512:(j + 1) * 512],
                                start=(i == 0), stop=(i == OD // P - 1),
                            )
                    if part == 1:
                        ev = o_pool.tile([P, 1024], F16, tag="ev")
                        nc.vector.tensor_scalar_add(
                            ev[:], ps[:], bo_sb[:, m:m + 1])
                        nc.sync.dma_start(
                            out=out_t[m * P:(m + 1) * P, csl], in_=ev[:])
                units.append(u)
            return units

        fillers = []   # drained inside attention blocks

        def attention_block(mt, h, qh, jit_v=False, rate=4):
            """One (head, query-half) attention block, software pipelined.

            jit_v: emit V-projection units just-in-time (first block only).
            rate: drain one filler unit every `rate` chunks.
            """
            ro = 64 * (h % 2)
            av = av_pool.tile([P, 1024], F32, tag="av", name="av")
            es_tiles = {}
            for kc in range(NKC):
                if jit_v and kc + 1 < NKC:
                    for u in vproj_units(kc + 1):
                        u()
                lg = lg_pool.tile([P, 1024], F32, tag="lg", name="lg")
                for j in range(2):
                    nc.tensor.matmul(
                        lg[:, j * 512:(j + 1) * 512],
                        kP[mt][ro:ro + 64, kc * P:(kc + 1) * P],
                        qT[mt][ro:ro + 64, qh * 1024 + j * 512:
                               qh * 1024 + (j + 1) * 512],
                        start=True, stop=True,
                    )
                es = e_pool.tile([P, 1024], F16, tag="es")
                nc.scalar.activation(
                    es[:], lg[:], AF.Exp,
                    bias=mk_sb[:, kc:kc + 1], scale=0.125,
                )
                es_tiles[kc] = es
                # AV for the previous chunk (keeps PE ahead of ACT)
                if kc >= 1:
                    _av_mm(av, h, kc - 1, es_tiles.pop(kc - 1))
                if (not jit_v) and fillers and kc % rate == rate - 1:
                    fillers.pop(0)[1]()
            _av_mm(av, h, NKC - 1, es_tiles.pop(NKC - 1), last=True)
            # free the av psum quickly (single copy), normalize from SBUF.
            # NB: reciprocal_approx_fast (custom DVE op) only works on APs
            # based at partition 0 — stage the denominator row there first.
            avs = s_pool.tile([65, 1024], F32, tag="avs")
            den = n_pool.tile([1, 1024], F32, tag="dn")
            rec = n_pool.tile([1, 1024], F32, tag="r0")
            bc = n_pool.tile([64, 1024], F32, tag="bc")
            nc.vector.tensor_copy(avs[:], av[0:65, :])
            nc.vector.tensor_copy(den[:], avs[64:65, :])
            nc.vector.reciprocal_approx_fast(rec[:], den[:])
            nc.gpsimd.partition_broadcast(bc[:], rec[:])
            nc.vector.tensor_mul(
                attn[qh][mt][ro:ro + 64, :], avs[0:64, :], bc[:])

        def _av_mm(av, h, kc, es, last=False):
            va_flat = v_aug[kc][:].rearrange("p h d -> p (h d)")
            for j in range(2):
                nc.tensor.matmul(
                    av[:, j * 512:(j + 1) * 512],
                    va_flat[:, 65 * h:65 * h + 128],
                    es[:, j * 512:(j + 1) * 512],
                    start=(kc == 0), stop=last,
                )

        # ---------- prologue, ordered by DMA arrival ----------
        # K n=0 groups need only wk + mem n=0 (the first bytes to land);
        # run them all while the rest of the inputs stream in.
        for m in range(OD // P):
            for u in kproj_units(m, ns=(0,)):
                u()
        # kP[0] cols n=1 are consumed by the first block's kc >= 8
        for u in kproj_units(0, ns=(1,)):
            u()
        for u in qproj_units(0, ns=(0,)):
            u()
        for u in vproj_units(0):
            u()

        # ---------- attention blocks ----------
        # first block carries the V projection just-in-time
        attention_block(0, 0, 0, jit_v=True)
        # queue K/Q for later head-pairs as fillers; tag = the (mt, qh)
        # key before which the unit must have been emitted (barrier below)
        for u in qproj_units(0, ns=(1,)):
            fillers.append((1, u))
        for m in range(1, OD // P):
            for u in kproj_units(m, ns=(1,)):
                fillers.append((2 * m, u))
            for u in qproj_units(m, ns=(0,)):
                fillers.append((2 * m, u))
            for u in qproj_units(m, ns=(1,)):
                fillers.append((2 * m + 1, u))

        order = []
        for mt in range(OD // P):
            for qh in range(2):
                for h in (2 * mt, 2 * mt + 1):
                    if (mt, h, qh) != (0, 0, 0):
                        order.append((mt, h, qh))

        for mt, h, qh in order:
            # barrier: everything this block needs must be emitted first
            while fillers and fillers[0][0] <= 2 * mt + qh:
                fillers.pop(0)[1]()
            # out-proj n=0 becomes available once all qh=0 attn written
            if (mt, h, qh) == (3, 6, 1):
                for m in range(D // P):
                    for u in oproj_units(m, 0):
                        fillers.append((4, u))
            rate = 2 if (mt, qh) == (3, 1) else (4 if fillers else NKC + 1)
            attention_block(mt, h, qh, rate=rate)

        # ---------- epilogue ----------
        # drain leftovers, then out-proj n=1; rotate groups across the
        # three free psum pools, part0s first, so earlier groups' matmuls
        # cover the final normalize chain and each psum->ev->store close
        while fillers:
            fillers.pop(0)[1]()
        pools = [pj_pool, lg_pool, av_pool]
        groups = [oproj_units(m, 1, pool=pools[m % 3])
                  for m in range(D // P)]
        for g in range(0, D // P, 3):
            trio = groups[g:g + 3]
            for t in trio:
                t[0]()
            for t in trio:
                t[1]()

    nc.compile()
    return nc


def _prep_inputs(x, memory, mask, wq, bq, wk, bk, wv, bv, wo, bo):
    f = np.float32
    h = np.float16
    wqT = np.ascontiguousarray(wq.T, dtype=f)
    wkT = np.ascontiguousarray(wk.T, dtype=f)
    wvT = np.ascontiguousarray(wv.T, dtype=f)
    woT = np.ascontiguousarray(wo.T, dtype=f)
    bo_eff = (bo.astype(f) + wo.astype(f) @ bv.astype(f))
    zeros_bo = np.zeros_like(bo_eff)
    in_maps = []
    for c in range(NCORES):
        b, g = divmod(c, 2)
        sl = slice(g * OD, (g + 1) * OD)
        bo_c = bo_eff if g == 0 else zeros_bo
        in_maps.append({
            "x_t": np.ascontiguousarray(x[b].T, dtype=h),
            "mem_t": np.ascontiguousarray(memory[b].T, dtype=h),
            "wq_t": np.ascontiguousarray(wqT[:, sl]).astype(h),
            "wk_t": np.ascontiguousarray(wkT[:, sl]).astype(h),
            "wv_t": np.ascontiguousarray(wvT[:, sl]).astype(h),
            "wo_t": np.ascontiguousarray(woT[sl, :]).astype(h),
            "bq_s": np.ascontiguousarray(bq[sl].astype(f).reshape(OD // P, P).T),
            "bk_s": np.ascontiguousarray(bk[sl].astype(f).reshape(OD // P, P).T),
            "bo_s": np.ascontiguousarray(bo_c.reshape(D // P, P).T),
            "maskb": np.ascontiguousarray(
                np.where(mask[b], np.float32(NEG), np.float32(0.0))
                .astype(f).reshape(NKC, P).T),
        })
    return in_maps


def kernel(x, memory, mask, wq, bq, wk, bk, wv, bv, wo, bo, **run_kwargs):
    x = np.asarray(x, dtype=np.float32)
    memory = np.asarray(memory, dtype=np.float32)
    mask = np.asarray(mask)
    if "nc" not in _cache:
        _cache["nc"] = _build()
    nc = _cache["nc"]
    in_maps = _prep_inputs(x, memory, mask, wq, bq, wk, bk, wv, bv, wo, bo)
    res = run_bass_kernel_spmd(nc, in_maps, list(range(NCORES)), **run_kwargs)
    out = np.empty((B, S, D), dtype=np.float32)
    for b in range(B):
        part = (res.results[2 * b]["out_t"].astype(np.float32)
                + res.results[2 * b + 1]["out_t"].astype(np.float32))
        out[b] = part.T
    if run_kwargs:
        _cache["last_results"] = res
    return out
